# revision 4
# baseline (speedup 1.0000x reference)
"""Trainium2 Bass kernel v3 for nn_Classifier_64587718197982 (spiking CNN).

HW-measured design constraints (axon-tunneled trn2, slope-benched):
  - fp16/fp8 single-rate matmuls run at roofline (~211ns per [C,512]);
    DoubleRow carries +120ns LDWEIGHTS overhead -> NOT used.
  - GpSimd(Pool) tensor_scalar is ~15us/op (avoid!); Pool tensor_tensor
    ~2.2us; DVE ops ~0.8-1.1us; Act activation ~0.9us. All on [C,1024].
  - Mixed-dtype matmuls (fp16 stationary x fp8 moving) are exact.

Design:
  - No DRAM round-trip: y stays in SBUF (y_hi fp16 + e5m2 lo for L1).
  - L1 conv: per tap, fp16(w) x fp16(x) + fp16(w) x e5m2(x - fp16(x)).
  - L2/3 conv consume LIF "gates" g = 0.25*(1-spike) in e4m3 directly:
    per tap fp16(-4w) x g, plus one "K-term" matmul (conv of all-ones,
    9-partition stationary wsum x indicator) per half. PSUM scale 1.
  - TDBN stats: drain accum (Act) gives sum(y); DVE stt (psum*1)*y_hi
    gives sum(y^2); tiny [C,2] AllReduce -> th' = 0.5/s, dp = d/s.
  - u-space LIF: u = g_prev*u + (y + dp); g = (u <= th')*0.25 on DVE.
    Chains per (t,b) on [C,1024]: DVE mult + add(+dp via stt for L1),
    Pool takes the (yhi+ylo) combine (L1) / plain adds b2-3 (L2/3).
"""
import numpy as np
import ml_dtypes
from contextlib import ExitStack

import concourse.bass as bass
import concourse.mybir as mybir
import concourse.tile as tile
from concourse import bass_isa
from concourse import bacc
from concourse.bass_utils import run_bass_kernel_spmd

F32 = mybir.dt.float32
FP16 = mybir.dt.float16
E4M3 = mybir.dt.float8e4
E5M2 = mybir.dt.float8e5
AF = mybir.ActivationFunctionType
ALU = mybir.AluOpType

T, B, C, H, W = 8, 32, 128, 32, 32
NCORES = 8
BL = B // NCORES
HW = H * W
NIMG = T * BL
NHALF = 2
RH = H // NHALF
DECAY = 0.25
THRESH = 0.5
BN_EPS = 1e-5
NCOUNT = float(T * HW)
GROUP = 3
DEBUG = False
SIM1 = False
NOAR = False
YLO1 = True               # keep e5m2 lo for L1's y (precision margin)
XLO_TERM = False          # e5m2 x-correction term in L1 conv

SHIFTS = [(1, 1)] + [(dy, dx) for dy in range(3) for dx in range(3)
                     if not (dy == 1 and dx == 1)]


def _tap_ranges(k, r_base):
    dy, dx = k
    oy, ox = dy - 1, dx - 1
    r0 = max(r_base, -oy)
    r1 = min(r_base + RH, H - oy)
    c0 = max(0, -ox)
    c1 = min(W, W - ox)
    return oy, ox, r0, r1, c0, c1


def build():
    nc = bacc.Bacc("TRN2", target_bir_lowering=False, debug=False,
                   num_devices=1 if SIM1 else NCORES)

    xm_d = nc.dram_tensor("xm", [T, BL, C, HW], FP16, kind="ExternalInput")
    xlo_d = nc.dram_tensor("xlo", [T, BL, C, HW], E5M2, kind="ExternalInput")
    wm1_d = nc.dram_tensor("wm1", [C, 9, C], FP16, kind="ExternalInput")
    w4_d, wk_d = {}, {}
    for l in (2, 3):
        w4_d[l] = nc.dram_tensor(f"w4{l}", [C, 9, C], FP16,
                                 kind="ExternalInput")
        wk_d[l] = nc.dram_tensor(f"wk{l}", [9, C], FP16, kind="ExternalInput")
    ind_d = nc.dram_tensor("ind", [9, 2 * RH * W], FP16, kind="ExternalInput")
    bn_d = {}
    for l in (1, 2, 3):
        bn_d[(l, "w")] = nc.dram_tensor(f"bnw{l}", [C, 1], F32,
                                        kind="ExternalInput")
        bn_d[(l, "b")] = nc.dram_tensor(f"bnb{l}", [C, 1], F32,
                                        kind="ExternalInput")
    fcw_d = nc.dram_tensor("fcw", [C, 10], F32, kind="ExternalInput")
    fcb_d = nc.dram_tensor("fcb", [1, 10], F32, kind="ExternalInput")
    out_d = nc.dram_tensor("out", [1, BL * 10], F32, kind="ExternalOutput")

    cc_bufs = {}
    for l in (1, 2, 3):
        cc_bufs[l] = (
            nc.dram_tensor(f"cc_in{l}", [C, 2], F32),
            nc.dram_tensor(f"cc_out{l}", [C, 2], F32, addr_space="Shared"),
        )

    with ExitStack() as ctx:
        tc = ctx.enter_context(tile.TileContext(nc))
        sb1 = ctx.enter_context(tc.tile_pool(name="sb1", bufs=1))
        yd_pool = ctx.enter_context(tc.tile_pool(name="yd", bufs=5))
        a3_pool = ctx.enter_context(tc.tile_pool(name="a3", bufs=2))
        scr_pool = ctx.enter_context(tc.tile_pool(name="scr", bufs=1))
        sq_pool = ctx.enter_context(tc.tile_pool(name="sq", bufs=2))
        mem_pool = ctx.enter_context(tc.tile_pool(name="mem", bufs=1))
        psum_pool = ctx.enter_context(
            tc.tile_pool(name="psum", bufs=8, space="PSUM"))

        # --- weights/constants ---
        wm1 = sb1.tile([C, 9, C], FP16)
        nc.sync.dma_start(out=wm1, in_=wm1_d[:, :, :])
        w4, wk = {}, {}
        for l in (2, 3):
            w4[l] = sb1.tile([C, 9, C], FP16, name=f"w4{l}")
            nc.sync.dma_start(out=w4[l], in_=w4_d[l][:, :, :])
            wk[l] = sb1.tile([9, C], FP16, name=f"wk{l}")
            nc.sync.dma_start(out=wk[l], in_=wk_d[l][:, :])
        ind = sb1.tile([9, 2, RH * W], FP16)
        nc.sync.dma_start(
            out=ind, in_=ind_d.rearrange("p (h n) -> p h n", h=2)[:, :, :])
        bn_sb = {}
        for key, dt_ in bn_d.items():
            bn_sb[key] = sb1.tile([C, 1], F32, name=f"bn{key[1]}{key[0]}")
            nc.sync.dma_start(out=bn_sb[key], in_=dt_[:, :])
        fcw_sb = sb1.tile([C, 10], F32)
        nc.sync.dma_start(out=fcw_sb, in_=fcw_d[:, :])
        fcb_sb = sb1.tile([1, 10], F32)
        nc.sync.dma_start(out=fcb_sb, in_=fcb_d[:, :])
        eps_t = sb1.tile([C, 1], F32)
        nc.vector.memset(eps_t, BN_EPS)

        # --- persistent activations ---
        yhi = sb1.tile([C, NIMG, HW], FP16)          # 64 KB/part
        gateA = sb1.tile([C, NIMG, HW], E4M3)        # 32 KB/part
        poolcnt = sb1.tile([C, T, BL], F32)
        statsA = {l: sb1.tile([C, NIMG * NHALF], F32, name=f"stA{l}")
                  for l in (1, 2, 3)}
        statsB = {l: sb1.tile([C, NIMG * NHALF], F32, name=f"stB{l}")
                  for l in (1, 2, 3)}

        def drain_stats(pt, img, h, l, lo_buf):
            slot = img * NHALF + h
            dst = yhi[:, img, h * RH * W:(h + 1) * RH * W]
            nc.scalar.activation(dst, pt, AF.Identity, bias=0.0, scale=1.0,
                                 accum_out=statsA[l][:, slot:slot + 1])
            if lo_buf is not None:
                nc.vector.scalar_tensor_tensor(
                    lo_buf[:, img, h * RH * W:(h + 1) * RH * W],
                    pt, 1.0, dst, op0=ALU.mult, op1=ALU.subtract)
            sqg = sq_pool.tile([C, RH * W], FP16, tag="sqg")
            nc.vector.scalar_tensor_tensor(
                sqg, pt, 1.0, dst, op0=ALU.mult, op1=ALU.mult,
                accum_out=statsB[l][:, slot:slot + 1])

        # =============== conv layer 1 (ylo scope) ===============
        ylo_ctx = tc.tile_pool(name="ylop", bufs=1)
        ylo_pool = ylo_ctx.__enter__()
        ylo = (ylo_pool.tile([C, NIMG, HW], E5M2, name="ylo")
               if YLO1 else None)
        with tc.tile_pool(name="xin", bufs=2 * GROUP) as xin_pool, \
                nc.named_scope("conv1"):
            for g0 in range(0, NIMG, GROUP):
                imgs = list(range(g0, min(g0 + GROUP, NIMG)))
                xm3, xlo3 = {}, {}
                for img in imgs:
                    t, b = divmod(img, BL)
                    xm_t = xin_pool.tile([C, HW], FP16, tag="xm")
                    nc.sync.dma_start(out=xm_t, in_=xm_d[t, b, :, :])
                    xm3[img] = xm_t.rearrange("c (h w) -> c h w", h=H)
                    xlo_t = xin_pool.tile([C, HW], E5M2, tag="xlo")
                    nc.sync.dma_start(out=xlo_t, in_=xlo_d[t, b, :, :])
                    xlo3[img] = xlo_t.rearrange("c (h w) -> c h w", h=H)
                pts = {}
                for img in imgs:
                    for h in range(NHALF):
                        pts[(img, h)] = psum_pool.tile([C, RH * W], F32,
                                                       name="pt")
                n_taps = len(SHIFTS)
                for ki, k in enumerate(SHIFTS):
                    kk = (k[0]) * 3 + (k[1])
                    for h in range(NHALF):
                        r_base = h * RH
                        oy, ox, r0, r1, c0, c1 = _tap_ranges(k, r_base)
                        for img in imgs:
                            p3 = pts[(img, h)].rearrange(
                                "c (r w) -> c r w", r=RH)
                            out_ap = p3[:, r0 - r_base:r1 - r_base, c0:c1]
                            nc.tensor.matmul(
                                out_ap, wm1[:, kk, :],
                                xm3[img][:, r0 + oy:r1 + oy, c0 + ox:c1 + ox],
                                start=(ki == 0),
                                stop=(not XLO_TERM and ki == n_taps - 1))
                            if XLO_TERM:
                                nc.tensor.matmul(
                                    out_ap, wm1[:, kk, :],
                                    xlo3[img][:, r0 + oy:r1 + oy,
                                              c0 + ox:c1 + ox],
                                    start=False, stop=(ki == n_taps - 1))
                for img in imgs:
                    for h in range(NHALF):
                        drain_stats(pts[(img, h)], img, h, 1, ylo)

        # =============== stats + AllReduce ===============
        def layer_stats(l):
            sA = sb1.tile([C, 1], F32, tag=f"sA{l}")
            nc.vector.tensor_reduce(sA, statsA[l], axis=mybir.AxisListType.X,
                                    op=ALU.add)
            sB = sb1.tile([C, 1], F32, tag=f"sB{l}")
            nc.vector.tensor_reduce(sB, statsB[l], axis=mybir.AxisListType.X,
                                    op=ALU.add)
            cc = sb1.tile([C, 2], F32, tag=f"cc{l}")
            nc.vector.tensor_copy(out=cc[:, 0:1], in_=sA)
            nc.vector.tensor_copy(out=cc[:, 1:2], in_=sB)
            cc_in, cc_out = cc_bufs[l]
            nc.sync.dma_start(out=cc_in[:, :], in_=cc)
            if not (SIM1 or NOAR):
                nc.gpsimd.collective_compute(
                    "AllReduce", ALU.add,
                    replica_groups=[list(range(NCORES))],
                    ins=[cc_in[:, :]], outs=[cc_out[:, :]],
                )
            ccr = sb1.tile([C, 2], F32, tag=f"ccr{l}")
            nc.sync.dma_start(out=ccr, in_=cc_in[:, :] if (SIM1 or NOAR)
                              else cc_out[:, :])
            n_tot = float(NIMG * HW * (1 if (SIM1 or NOAR) else NCORES))
            mean = sb1.tile([C, 1], F32, tag=f"mean{l}")
            nc.vector.tensor_scalar(mean, ccr[:, 0:1], 1.0 / n_tot, None,
                                    op0=ALU.mult)
            ex2 = sb1.tile([C, 1], F32, tag=f"ex2{l}")
            nc.vector.tensor_scalar(ex2, ccr[:, 1:2], 1.0 / n_tot, None,
                                    op0=ALU.mult)
            var = sb1.tile([C, 1], F32, tag=f"var{l}")
            nc.vector.tensor_tensor(var, mean, mean, op=ALU.mult)
            nc.vector.tensor_tensor(var, ex2, var, op=ALU.subtract)
            sd = sb1.tile([C, 1], F32, tag=f"sd{l}")
            nc.scalar.activation(sd, var, AF.Sqrt, bias=eps_t, scale=1.0)
            rb = sb1.tile([C, 1], F32, tag=f"rb{l}")
            nc.vector.reciprocal(out=rb, in_=bn_sb[(l, "w")])
            inv_s = sb1.tile([C, 1], F32, tag=f"invs{l}")
            nc.vector.tensor_tensor(inv_s, sd, rb, op=ALU.mult)
            thp = sb1.tile([C, 1], F32, tag=f"thp{l}")
            nc.vector.tensor_scalar(thp, inv_s, THRESH, None, op0=ALU.mult)
            dp = sb1.tile([C, 1], F32, tag=f"dp{l}")
            nc.vector.tensor_tensor(dp, bn_sb[(l, "b")], inv_s, op=ALU.mult)
            nc.vector.tensor_tensor(dp, dp, mean, op=ALU.subtract)
            return thp, dp

        # =============== LIF ===============
        umem = mem_pool.tile([C, BL * HW], F32, tag="umem")
        u3 = umem.rearrange("c (b p) -> c b p", b=BL)

        def lif_layer(l, thp, dp, gates):
            """g = 0.25*(u<=th') e4m3 on DVE (conv rhs AND membrane
            multiplier)."""
            prev = [None] * BL
            for t in range(T):
                for b in range(BL):
                    img = t * BL + b
                    yh = yhi[:, img, :]
                    if l == 1 and YLO1:
                        dst = yd_pool.tile([C, HW], F32, tag="yd")
                        nc.gpsimd.tensor_tensor(dst, yh, ylo[:, img, :],
                                                op=ALU.add)
                        if t == 0:
                            nc.vector.tensor_scalar(u3[:, b], dst, dp, None,
                                                    op0=ALU.add)
                        else:
                            nc.vector.tensor_tensor(u3[:, b], u3[:, b],
                                                    prev[b], op=ALU.mult)
                            nc.vector.scalar_tensor_tensor(
                                u3[:, b], dst, dp, u3[:, b],
                                op0=ALU.add, op1=ALU.add)
                    else:
                        if t == 0:
                            nc.scalar.activation(u3[:, b], yh, AF.Identity,
                                                 bias=dp, scale=1.0)
                        else:
                            dst = yd_pool.tile([C, HW], F32, tag="yd")
                            nc.scalar.activation(dst, yh, AF.Identity,
                                                 bias=dp, scale=1.0)
                            nc.vector.tensor_tensor(u3[:, b], u3[:, b],
                                                    prev[b], op=ALU.mult)
                            aeng = nc.gpsimd if b >= 1 else nc.vector
                            aeng.tensor_tensor(u3[:, b], u3[:, b], dst,
                                               op=ALU.add)
                    if gates is not None:
                        g = gates[:, img, :]
                    else:
                        g = a3_pool.tile([C, HW], E4M3, tag="a3")
                    nc.vector.tensor_scalar(g, u3[:, b], thp, DECAY,
                                            op0=ALU.is_le, op1=ALU.mult)
                    if gates is None:
                        scr = scr_pool.tile([C, HW], FP16, tag="scr")
                        nc.scalar.activation(
                            scr, g, AF.Identity, bias=0.0, scale=1.0,
                            accum_out=poolcnt[:, t, b:b + 1])
                    prev[b] = g

        with nc.named_scope("stats1"):
            thp1, dp1 = layer_stats(1)
        with nc.named_scope("lif1"):
            lif_layer(1, thp1, dp1, gateA)
        ylo_ctx.__exit__(None, None, None)
        gateB_pool = ctx.enter_context(tc.tile_pool(name="gBp", bufs=1))
        gateB = gateB_pool.tile([C, NIMG, HW], E4M3)

        # =============== conv layers 2/3 ===============
        def conv_l23(l, gates):
            g3 = gates.rearrange("c n (h w) -> c n h w", h=H)
            for g0 in range(0, NIMG, BL):
                imgs = list(range(g0, g0 + BL))
                pts = {}
                for img in imgs:
                    for h in range(NHALF):
                        pt = psum_pool.tile([C, RH * W], F32, name="pt")
                        pts[(img, h)] = pt
                        nc.tensor.matmul(pt, wk[l][:, :], ind[:, h, :],
                                         start=True, stop=False)
                n_taps = len(SHIFTS)
                for ki, k in enumerate(SHIFTS):
                    kk = (k[0]) * 3 + (k[1])
                    for h in range(NHALF):
                        r_base = h * RH
                        oy, ox, r0, r1, c0, c1 = _tap_ranges(k, r_base)
                        for img in imgs:
                            p3 = pts[(img, h)].rearrange(
                                "c (r w) -> c r w", r=RH)
                            out_ap = p3[:, r0 - r_base:r1 - r_base, c0:c1]
                            nc.tensor.matmul(
                                out_ap, w4[l][:, kk, :],
                                g3[:, img, r0 + oy:r1 + oy, c0 + ox:c1 + ox],
                                start=False, stop=(ki == n_taps - 1))
                for img in imgs:
                    for h in range(NHALF):
                        drain_stats(pts[(img, h)], img, h, l, None)

        with nc.named_scope("conv2"):
            conv_l23(2, gateA)
        with nc.named_scope("stats2"):
            thp2, dp2 = layer_stats(2)
        with nc.named_scope("lif2"):
            lif_layer(2, thp2, dp2, gateB)
        with nc.named_scope("conv3"):
            conv_l23(3, gateB)
        with nc.named_scope("stats3"):
            thp3, dp3 = layer_stats(3)
        with nc.named_scope("lif3"):
            lif_layer(3, thp3, dp3, None)

        # =============== head ===============
        feat = sb1.tile([C, BL], F32)
        for b in range(BL):
            nc.vector.tensor_reduce(feat[:, b:b + 1], poolcnt[:, :, b],
                                    axis=mybir.AxisListType.X, op=ALU.add)
        nc.vector.tensor_scalar(feat, feat, -4.0 / NCOUNT, 1.0,
                                op0=ALU.mult, op1=ALU.add)
        prod = sb1.tile([C, BL, 10], F32)
        nc.vector.tensor_tensor(
            prod, feat.unsqueeze(2).broadcast_to([C, BL, 10]),
            fcw_sb.unsqueeze(1).broadcast_to([C, BL, 10]), op=ALU.mult)
        red = sb1.tile([C, BL, 10], F32)
        nc.gpsimd.partition_all_reduce(red, prod, channels=C,
                                       reduce_op=bass_isa.ReduceOp.add)
        ofin = sb1.tile([1, BL, 10], F32)
        nc.vector.tensor_tensor(
            ofin, red[0:1],
            fcb_sb.unsqueeze(1).broadcast_to([1, BL, 10]), op=ALU.add)
        nc.sync.dma_start(out=out_d[:, :],
                          in_=ofin.rearrange("c b k -> c (b k)"))

    nc.compile()
    return nc


_NC_CACHE = {}


def _get_nc():
    if "nc" not in _NC_CACHE:
        _NC_CACHE["nc"] = build()
    return _NC_CACHE["nc"]


def make_in_maps(inp, conv_ws, bns, fc_w, fc_b):
    common = {}
    w1, w2, w3 = conv_ws
    w1t = np.ascontiguousarray(w1.transpose(1, 2, 3, 0).reshape(C, 9, C))
    common["wm1"] = w1t.astype(np.float16)
    for l, w in ((2, w2), (3, w3)):
        wt = np.ascontiguousarray(w.transpose(1, 2, 3, 0).reshape(C, 9, C))
        common[f"w4{l}"] = (-4.0 * wt).astype(np.float16)
        common[f"wk{l}"] = np.ascontiguousarray(
            wt.sum(axis=0).astype(np.float16))          # [9, C]
    ind = np.zeros((9, 2, RH, W), np.float16)
    for dy in range(3):
        for dx in range(3):
            oy, ox = dy - 1, dx - 1
            j = dy * 3 + dx
            for h in range(2):
                for rr in range(RH):
                    r = h * RH + rr
                    if 0 <= r + oy < H:
                        c_lo, c_hi = max(0, -ox), min(W, W - ox)
                        ind[j, h, rr, c_lo:c_hi] = 1.0
    common["ind"] = np.ascontiguousarray(ind.reshape(9, 2 * RH * W))
    for li in (1, 2, 3):
        common[f"bnw{li}"] = np.ascontiguousarray(
            bns[li - 1][0].reshape(C, 1).astype(np.float32))
        common[f"bnb{li}"] = np.ascontiguousarray(
            bns[li - 1][1].reshape(C, 1).astype(np.float32))
    common["fcw"] = np.ascontiguousarray(fc_w.T.astype(np.float32))
    common["fcb"] = np.ascontiguousarray(fc_b.reshape(1, 10).astype(np.float32))

    E5 = ml_dtypes.float8_e5m2
    in_maps = []
    for cid in range(NCORES):
        xc = np.ascontiguousarray(
            inp[:, cid * BL:(cid + 1) * BL].reshape(T, BL, C, HW))
        xm = xc.astype(np.float16)
        xlo = (xc - xm.astype(np.float32)).astype(E5)
        m = dict(common)
        m["xm"] = xm
        m["xlo"] = xlo
        in_maps.append(m)
    return in_maps


def kernel(inp, conv_w1, conv_w2, conv_w3, bn_w1, bn_b1, bn_w2, bn_b2,
           bn_w3, bn_b3, fc_w, fc_b):
    inp = np.asarray(inp, dtype=np.float32)
    ws = [np.asarray(w, dtype=np.float32) for w in (conv_w1, conv_w2, conv_w3)]
    bns = [(np.asarray(bn_w1, np.float32), np.asarray(bn_b1, np.float32)),
           (np.asarray(bn_w2, np.float32), np.asarray(bn_b2, np.float32)),
           (np.asarray(bn_w3, np.float32), np.asarray(bn_b3, np.float32))]
    fc_w = np.asarray(fc_w, np.float32)
    fc_b = np.asarray(fc_b, np.float32)

    nc = _get_nc()
    in_maps = make_in_maps(inp, ws, bns, fc_w, fc_b)
    res = run_bass_kernel_spmd(nc, in_maps, core_ids=list(range(NCORES)))
    out = np.concatenate(
        [r["out"].reshape(BL, 10) for r in res.results], axis=0)
    return out.astype(np.float32)
